# revision 3
# baseline (speedup 1.0000x reference)
"""Trainium2 Bass kernel v2 for a Llama block (B=2, S=2048, D=2048, H=16,
FF=8192) — head-sharded causal attention.

Sharding (8 cores, static SPMD):
  - core c owns heads {2c, 2c+1} over BOTH batches for Q/K/V projection and
    attention.  Attention is causal-triangular: query block qb (512 tokens)
    only visits key blocks 0..4qb+3; only the 4 diagonal key blocks get a
    (multiplicative, post-exp) mask.  No K/V AllGather at all.
  - attention outputs are redistributed with two AllToAlls over [[0..7]]
    (one per local head, issued as soon as that head's attention is done)
    so core c ends up with all 16 heads for its own 512-token chunk.
  - WO + residual + norm2 + FFN are token-sharded: core c owns tokens
    [c*512, (c+1)*512) of the flattened (b, s) axis.
  - activations live in [feature, token] layout throughout; matmuls in bf16;
    RMSNorm weights folded into the following projection weights on the host.
"""

import math
import os
import sys

sys.path.insert(0, "/opt/trn_rl_repo")

import ml_dtypes
import numpy as np

import concourse.bass as bass
import concourse.mybir as mybir
import concourse.tile as tile
from concourse import bacc
from concourse.bass_utils import run_bass_kernel_spmd

F32 = mybir.dt.float32
BF16 = mybir.dt.bfloat16
F8 = mybir.dt.float8e4
AFT = mybir.ActivationFunctionType
DR = mybir.MatmulPerfMode.DoubleRow
WSC = 64.0              # fp8 weight scale for FFN weights

B, S, D, H = 2, 2048, 2048, 16
HD = D // H            # 128
FF = 4 * D             # 8192
NC = 8
TOK = 512              # own tokens per core (FFN phase)
NH = 2                 # heads per core
EPS = 1e-6
BASE = 10000.0
P = 128
DCH = D // P           # 16 d-chunks
FCH = FF // P          # 64 ff subchunks
NTB = (B * S) // TOK   # 8 token blocks over both batches
SCALE = 1.0 / math.sqrt(HD)

_CACHE = {}
LAST_RESULT = None


def _rope_tables():
    """[128, S] cos/sin tables with the 64-row table duplicated in both
    partition halves (for lane-aligned rope on-device)."""
    t = BASE ** (-2.0 * (np.arange(HD // 2, dtype=np.float64) - 1.0) / HD)
    ang = np.arange(S, dtype=np.float64)[:, None] * t[None, :]   # [S, 64]
    c = np.cos(ang).T.astype(np.float32)
    sn = np.sin(ang).T.astype(np.float32)
    return (np.concatenate([c, c], axis=0), np.concatenate([sn, sn], axis=0))


def _build_program():
    nc = bacc.Bacc("TRN2", target_bir_lowering=False, debug=False,
                   num_devices=NC)

    def inp(name, shape, dtype=F32):
        return nc.dram_tensor(name, shape, dtype, kind="ExternalInput").ap()

    xT_all = inp("xT_all", [D, B * S])     # full x, both batches, transposed
    xT_own = inp("xT_own", [D, TOK])       # own token chunk, transposed
    wq = inp("wq", [P, DCH, NH * HD], BF16)   # [p, o, f] for own 2 heads
    wk = inp("wk", [P, DCH, NH * HD], BF16)
    wv = inp("wv", [P, DCH, NH * HD], BF16)
    wo = inp("wo", [DCH, P, DCH, P], BF16)    # [o, p, hc, f]
    wg = inp("wg", [FCH, P, 8, 2, P], F8)     # [fb, p, op, j, f], x64
    wu = inp("wu", [FCH, P, 8, 2, P], F8)
    wd = inp("wd", [4, DCH, P, 8, 2, P], F8)  # [sc, o, p, fsp, j, f], x64
    bq = inp("bq", [P, NH], F32)
    bk = inp("bk", [P, NH], F32)
    bvb = inp("bvb", [P, NH * HD], F32)    # v-bias broadcast over partitions
    bo = inp("bo", [P, DCH], F32)
    bg = inp("bg", [P, FCH], F32)
    bu = inp("bu", [P, FCH], F32)
    bd = inp("bd", [P, DCH], F32)
    cosk = inp("cosk", [P, S], BF16)
    sink = inp("sink", [P, S], BF16)
    maskb = inp("maskb", [P, 4, TOK], BF16)   # 0/1 diagonal masks, post-exp
    onesb = inp("onesb", [P, 1], BF16)
    epsv = inp("epsv", [P, 1], F32)
    out_t = nc.dram_tensor("out", [D, TOK], F32, kind="ExternalOutput").ap()

    xT3 = xT_all.rearrange("(o p) t -> p o t", p=P)
    xT_own3 = xT_own.rearrange("(o p) t -> p o t", p=P)

    # AllToAll bounce buffers, one per local head (h2).  Block j rows hold
    # head 2j+h2's [HD, 512] slice for core j's token chunk.
    a2a_in = [nc.dram_tensor(f"a2a_in{h2}", [NC * HD, TOK], BF16).ap()
              for h2 in range(NH)]
    a2a_out = [nc.dram_tensor(f"a2a_out{h2}", [NC * HD, TOK], BF16).ap()
               for h2 in range(NH)]

    with tile.TileContext(nc) as tc:
        with tc.tile_pool(name="consts", bufs=1) as consts, \
             tc.tile_pool(name="res", bufs=1) as res:
            onesb_s = consts.tile([P, 1], BF16)
            nc.sync.dma_start(onesb_s[:], onesb[:])
            eps_s = consts.tile([P, 1], F32)
            nc.sync.dma_start(eps_s[:], epsv[:])
            bq_s = consts.tile([P, NH], F32)
            nc.sync.dma_start(bq_s[:], bq[:])
            bk_s = consts.tile([P, NH], F32)
            nc.sync.dma_start(bk_s[:], bk[:])
            bvb_s = consts.tile([P, NH * HD], F32)
            nc.sync.dma_start(bvb_s[:], bvb[:])
            bo_s = consts.tile([P, DCH], F32)
            nc.sync.dma_start(bo_s[:], bo[:])
            bg_s = consts.tile([P, FCH], F32)
            nc.sync.dma_start(bg_s[:], bg[:])
            bu_s = consts.tile([P, FCH], F32)
            nc.sync.dma_start(bu_s[:], bu[:])
            bd_s = consts.tile([P, DCH], F32)
            nc.sync.dma_start(bd_s[:], bd[:])
            mask_s = consts.tile([P, 4, TOK], BF16)
            nc.sync.dma_start(mask_s[:], maskb[:])

            acc = res.tile([P, DCH, TOK], F32)   # x2 accumulator (D..E)

            def rope(pool, src, cos_t, sin_t, dst, tname):
                """src [128, n] bf16 -> dst roped bf16; cos/sin [128, n]
                tables with duplicated halves (lane-aligned via a
                half-swapped copy)."""
                n = src.shape[-1]
                hh = HD // 2
                swp = pool.tile([P, n], BF16, tag="rpsw", name=f"{tname}sw")
                nc.sync.dma_start(swp[0:hh, :], src[hh:P, :])
                nc.sync.dma_start(swp[hh:P, :], src[0:hh, :])
                ma = pool.tile([P, n], BF16, tag="rp1", name=f"{tname}ma")
                mb = pool.tile([P, n], BF16, tag="rp2", name=f"{tname}mb")
                nc.vector.tensor_mul(out=ma[:], in0=src[:], in1=cos_t)
                nc.vector.tensor_mul(out=mb[:], in0=swp[:], in1=sin_t)
                nc.vector.tensor_add(out=dst[0:hh], in0=ma[0:hh],
                                     in1=mb[0:hh])
                nc.vector.tensor_sub(out=dst[hh:P], in0=mb[hh:P],
                                     in1=ma[hh:P])

            with tc.tile_pool(name="attres", bufs=1) as ares, \
                 tc.tile_pool(name="qkvw", bufs=1) as qkvw:
                wq_s = qkvw.tile([P, DCH, NH * HD], BF16)
                nc.sync.dma_start(wq_s[:], wq[:])
                wk_s = qkvw.tile([P, DCH, NH * HD], BF16)
                nc.sync.dma_start(wk_s[:], wk[:])
                wv_s = qkvw.tile([P, DCH, NH * HD], BF16)
                nc.sync.dma_start(wv_s[:], wv[:])
                cskt = qkvw.tile([P, 2, S], BF16)
                nc.sync.dma_start(cskt[:, 0, :], cosk[:])
                nc.sync.dma_start(cskt[:, 1, :], sink[:])

                # resident activations for attention
                qt_all = ares.tile([P, NH, NTB, TOK], BF16)  # [hd,h2,tb,tok]
                kt_all = ares.tile([P, NH, NTB, TOK], BF16)
                v_all = ares.tile([P, NH, NTB * 4, HD], BF16)  # [k,h2,kb,hd]

                # ---- Phase A: per token block, norm + Q/K/V own heads ----
                with tc.tile_pool(name="pa", bufs=2) as pool, \
                     tc.tile_pool(name="paps", bufs=2, space="PSUM") as psum:
                    for tb in range(NTB):
                        ss = tb % 4  # position block within the batch
                        xc = pool.tile([P, DCH, TOK], BF16, tag="xc",
                                       name=f"xc{tb}")
                        for half in range(2):
                            nc.gpsimd.dma_start(
                                xc[:, bass.ts(half, DCH // 2), :],
                                xT3[:, bass.ts(half, DCH // 2),
                                    bass.ts(tb, TOK)])
                        sumsq = psum.tile([1, TOK], F32, tag="ssps",
                                          name=f"ss{tb}")
                        for o in range(DCH):
                            sq = pool.tile([P, TOK], BF16, tag="sq",
                                           name=f"sq{tb}_{o}")
                            nc.scalar.activation(sq[:], xc[:, o, :],
                                                 AFT.Square)
                            nc.tensor.matmul(sumsq[:], lhsT=onesb_s[:],
                                             rhs=sq[:], start=(o == 0),
                                             stop=(o == DCH - 1))
                        rms = pool.tile([1, TOK], F32, tag="rms",
                                        name=f"rms{tb}")
                        nc.scalar.activation(rms[:], sumsq[:], AFT.Sqrt,
                                             scale=1.0 / D, bias=eps_s[:1])
                        rec = pool.tile([1, TOK], F32, tag="rec",
                                        name=f"rec{tb}")
                        nc.vector.reciprocal(rec[:], rms[:])
                        rbc = pool.tile([P, TOK], F32, tag="rbc",
                                        name=f"rbc{tb}")
                        nc.gpsimd.partition_broadcast(rbc[:], rec[:])
                        nx = pool.tile([P, DCH, TOK], BF16, tag="nx",
                                       name=f"nx{tb}")
                        for o in range(DCH):
                            nc.vector.tensor_mul(out=nx[:, o, :],
                                                 in0=xc[:, o, :], in1=rbc[:])
                        # Q and K projections + rope for the 2 own heads
                        for h2 in range(NH):
                            qp = psum.tile([P, TOK], F32, tag="qps",
                                           name=f"qps{tb}_{h2}")
                            kp = psum.tile([P, TOK], F32, tag="kps",
                                           name=f"kps{tb}_{h2}")
                            for o in range(DCH):
                                st, sp = (o == 0), (o == DCH - 1)
                                nc.tensor.matmul(
                                    qp[:], lhsT=wq_s[:, o, bass.ts(h2, HD)],
                                    rhs=nx[:, o, :], start=st, stop=sp)
                                nc.tensor.matmul(
                                    kp[:], lhsT=wk_s[:, o, bass.ts(h2, HD)],
                                    rhs=nx[:, o, :], start=st, stop=sp)
                            qb_t = pool.tile([P, TOK], BF16, tag="qbias",
                                             name=f"qb{tb}_{h2}")
                            nc.scalar.activation(qb_t[:], qp[:], AFT.Identity,
                                                 bias=bq_s[:, h2:h2 + 1])
                            rope(pool, qb_t[:], cskt[:, 0, bass.ts(ss, TOK)],
                                 cskt[:, 1, bass.ts(ss, TOK)],
                                 qt_all[:, h2, tb, :], f"qr{tb}_{h2}")
                            kb_t = pool.tile([P, TOK], BF16, tag="kbias",
                                             name=f"kb{tb}_{h2}")
                            nc.scalar.activation(kb_t[:], kp[:], AFT.Identity,
                                                 bias=bk_s[:, h2:h2 + 1])
                            rope(pool, kb_t[:], cskt[:, 0, bass.ts(ss, TOK)],
                                 cskt[:, 1, bass.ts(ss, TOK)],
                                 kt_all[:, h2, tb, :], f"kr{tb}_{h2}")
                        # V projection: [tokens, hd] layout per 128-tok block
                        for ts_ in range(4):
                            vp = psum.tile([P, NH * HD], F32, tag="vps",
                                           name=f"vps{tb}_{ts_}")
                            for o in range(DCH):
                                nc.tensor.matmul(
                                    vp[:], lhsT=nx[:, o, bass.ts(ts_, P)],
                                    rhs=wv_s[:, o, :], start=(o == 0),
                                    stop=(o == DCH - 1))
                            vsb = pool.tile([P, NH * HD], BF16, tag="vsb",
                                            name=f"vsb{tb}_{ts_}")
                            nc.vector.tensor_add(out=vsb[:], in0=vp[:],
                                                 in1=bvb_s[:])
                            for h2 in range(NH):
                                nc.vector.tensor_copy(
                                    v_all[:, h2, tb * 4 + ts_, :],
                                    vsb[:, bass.ts(h2, HD)])

                # ---- Phase B: causal attention for own heads ----
                # h2-major so each head's AllToAll can be issued early.
                with tc.tile_pool(name="pb", bufs=3) as pool, \
                     tc.tile_pool(name="pbst", bufs=4, space="PSUM") as psst, \
                     tc.tile_pool(name="pbps", bufs=2, space="PSUM") as psum:
                    for h2 in range(NH):
                        for b in range(B):
                            for qb in range(4):
                                tbq = b * 4 + qb
                                nkb = 4 * qb + 4
                                den = psum.tile([1, TOK], F32, tag="denps",
                                                name=f"den{h2}_{tbq}")
                                op = psum.tile([P, TOK], F32, tag="outps",
                                               name=f"op{h2}_{tbq}")
                                for kb in range(nkb):
                                    tbk = b * 4 + kb // 4
                                    cs = (kb % 4) * P
                                    stp = psst.tile(
                                        [P, TOK], F32, tag="stps",
                                        name=f"st{h2}_{tbq}_{kb}")
                                    nc.tensor.matmul(
                                        stp[:],
                                        lhsT=kt_all[:, h2, tbk,
                                                    bass.ds(cs, P)],
                                        rhs=qt_all[:, h2, tbq, :],
                                        start=True, stop=True)
                                    est = pool.tile(
                                        [P, TOK], BF16, tag="est",
                                        name=f"est{h2}_{tbq}_{kb}")
                                    dd = kb - 4 * qb
                                    if dd >= 0:
                                        epre = pool.tile(
                                            [P, TOK], BF16, tag="epre",
                                            name=f"ep{h2}_{tbq}_{kb}")
                                        nc.scalar.activation(
                                            epre[:], stp[:], AFT.Exp,
                                            scale=SCALE)
                                        nc.vector.tensor_mul(
                                            out=est[:], in0=epre[:],
                                            in1=mask_s[:, dd, :])
                                    else:
                                        nc.scalar.activation(
                                            est[:], stp[:], AFT.Exp,
                                            scale=SCALE)
                                    st, sp = (kb == 0), (kb == nkb - 1)
                                    nc.tensor.matmul(den[:], lhsT=onesb_s[:],
                                                     rhs=est[:], start=st,
                                                     stop=sp)
                                    nc.tensor.matmul(
                                        op[:],
                                        lhsT=v_all[:, h2, b * 16 + kb, :],
                                        rhs=est[:], start=st, stop=sp)
                                recd = pool.tile([1, TOK], F32, tag="recd",
                                                 name=f"recd{h2}_{tbq}")
                                nc.vector.reciprocal(recd[:], den[:])
                                rdb = pool.tile([P, TOK], F32, tag="rdb",
                                                name=f"rdb{h2}_{tbq}")
                                nc.gpsimd.partition_broadcast(rdb[:],
                                                              recd[:])
                                att = pool.tile([P, TOK], BF16, tag="att",
                                                name=f"att{h2}_{tbq}")
                                nc.vector.tensor_mul(out=att[:], in0=op[:],
                                                     in1=rdb[:])
                                nc.sync.dma_start(
                                    a2a_in[h2][bass.ds(tbq * P, P), :],
                                    att[:])
                        nc.gpsimd.collective_compute(
                            "AllToAll", mybir.AluOpType.bypass,
                            ins=[a2a_in[h2][:].opt()],
                            outs=[a2a_out[h2][:].opt()],
                            replica_groups=[list(range(NC))])

            # ---- Phase D: WO for own tokens + residual ----
            a2a3 = [a2a_out[h2].rearrange("(i p) t -> p i t", p=P)
                    for h2 in range(NH)]
            with tc.tile_pool(name="pd", bufs=2) as pool, \
                 tc.tile_pool(name="pdw", bufs=3) as wpool, \
                 tc.tile_pool(name="pdps", bufs=4, space="PSUM") as psum:
                attr = [None, None]
                for h2 in range(NH):
                    attr[h2] = pool.tile([P, NC, TOK], BF16, tag=f"attr{h2}",
                                         name=f"attr{h2}")
                    nc.sync.dma_start(attr[h2][:], a2a3[h2][:])
                for o in range(DCH):
                    wo_s = wpool.tile([P, DCH, P], BF16, tag="wos",
                                      name=f"wos{o}")
                    nc.sync.dma_start(wo_s[:], wo[o])
                    x2p = psum.tile([P, TOK], F32, tag="x2ps", name=f"x2p{o}")
                    # h2=0 chunks first: those arrive with the first AllToAll,
                    # so WO can start while the second is still in flight.
                    order = [(i, 0) for i in range(NC)] + \
                            [(i, 1) for i in range(NC)]
                    for n, (i, h2) in enumerate(order):
                        nc.tensor.matmul(x2p[:], lhsT=wo_s[:, 2 * i + h2, :],
                                         rhs=attr[h2][:, i, :],
                                         start=(n == 0), stop=(n == DCH - 1))
                    x2pre = pool.tile([P, TOK], F32, tag="x2pre",
                                      name=f"x2pre{o}")
                    nc.scalar.activation(x2pre[:], x2p[:], AFT.Identity,
                                         bias=bo_s[:, o:o + 1])
                    xres = pool.tile([P, TOK], F32, tag="xres",
                                     name=f"xres{o}")
                    nc.sync.dma_start(xres[:], xT_own3[:, o, :])
                    nc.vector.tensor_add(out=acc[:, o, :], in0=x2pre[:],
                                         in1=xres[:])

            # ---- Phase E: norm2 + FFN (streaming full weights) ----
            out3 = out_t.rearrange("(o p) t -> p o t", p=P)
            with tc.tile_pool(name="pe", bufs=2) as pool, \
                 tc.tile_pool(name="pew", bufs=3) as wpool, \
                 tc.tile_pool(name="peact", bufs=1) as actp, \
                 tc.tile_pool(name="peps", bufs=2, space="PSUM") as psum:
                nx2 = actp.tile([P, DCH, TOK], F8)
                act2 = actp.tile([P, 2, DCH, TOK], F8)
                rbc2 = pool.tile([P, TOK], F32, tag="rbc2")
                sumsq = psum.tile([1, TOK], F32, tag="n2ss")
                for o in range(DCH):
                    sq = pool.tile([P, TOK], BF16, tag="n2sq", name=f"n2sq{o}")
                    nc.scalar.activation(sq[:], acc[:, o, :], AFT.Square)
                    nc.tensor.matmul(sumsq[:], lhsT=onesb_s[:], rhs=sq[:],
                                     start=(o == 0), stop=(o == DCH - 1))
                rms = pool.tile([1, TOK], F32, tag="n2rms")
                nc.scalar.activation(rms[:], sumsq[:], AFT.Sqrt,
                                     scale=1.0 / D, bias=eps_s[:1])
                rec = pool.tile([1, TOK], F32, tag="n2rec")
                nc.vector.reciprocal(rec[:], rms[:])
                nc.gpsimd.partition_broadcast(rbc2[:], rec[:])
                with nc.allow_low_precision(reason="fp8 FFN activations"):
                    for o in range(DCH):
                        nc.vector.tensor_mul(out=nx2[:, o, :],
                                             in0=acc[:, o, :], in1=rbc2[:])
                # fold b_down into acc now (added once)
                for o in range(DCH):
                    nc.vector.tensor_scalar_add(acc[:, o, :], acc[:, o, :],
                                                bd_s[:, o:o + 1])
                for sc in range(4):
                    for fs in range(DCH):
                        f = sc * DCH + fs
                        wg_s = wpool.tile([P, 8, 2, P], F8, tag="wgs",
                                          name=f"wgs{f}")
                        nc.sync.dma_start(wg_s[:], wg[f])
                        wu_s = wpool.tile([P, 8, 2, P], F8, tag="wus",
                                          name=f"wus{f}")
                        nc.sync.dma_start(wu_s[:], wu[f])
                        gp = psum.tile([P, TOK], F32, tag="gps", name=f"gps{f}")
                        up = psum.tile([P, TOK], F32, tag="ups", name=f"ups{f}")
                        for o2 in range(DCH // 2):
                            st, sp = (o2 == 0), (o2 == DCH // 2 - 1)
                            nc.tensor.matmul(
                                gp[:], lhsT=wg_s[:, o2],
                                rhs=nx2[:, bass.ds(2 * o2, 2), :],
                                start=st, stop=sp, perf_mode=DR)
                            nc.tensor.matmul(
                                up[:], lhsT=wu_s[:, o2],
                                rhs=nx2[:, bass.ds(2 * o2, 2), :],
                                start=st, stop=sp, perf_mode=DR)
                        gs = pool.tile([P, TOK], F32, tag="gsig", name=f"gs{f}")
                        nc.scalar.activation(gs[:], gp[:], AFT.Silu,
                                             scale=1.0 / WSC,
                                             bias=bg_s[:, f:f + 1])
                        us = pool.tile([P, TOK], F32, tag="usig", name=f"us{f}")
                        nc.scalar.activation(us[:], up[:], AFT.Identity,
                                             scale=1.0 / WSC,
                                             bias=bu_s[:, f:f + 1])
                        with nc.allow_low_precision(reason="fp8 FFN act2"):
                            nc.vector.tensor_mul(out=act2[:, sc % 2, fs, :],
                                                 in0=gs[:], in1=us[:])
                    for o in range(DCH):
                        wd_s = wpool.tile([P, 8, 2, P], F8, tag="wds",
                                          name=f"wds{sc}_{o}")
                        nc.sync.dma_start(wd_s[:], wd[sc, o])
                        dp = psum.tile([P, TOK], F32, tag="dps",
                                       name=f"dps{sc}_{o}")
                        for fsp in range(DCH // 2):
                            nc.tensor.matmul(
                                dp[:], lhsT=wd_s[:, fsp],
                                rhs=act2[:, sc % 2, bass.ds(2 * fsp, 2), :],
                                start=(fsp == 0), stop=(fsp == DCH // 2 - 1),
                                perf_mode=DR)
                        dsc = pool.tile([P, TOK], F32, tag="dsc",
                                        name=f"dsc{sc}_{o}")
                        nc.scalar.activation(dsc[:], dp[:], AFT.Identity,
                                             scale=1.0 / WSC)
                        nc.vector.tensor_add(out=acc[:, o, :],
                                             in0=acc[:, o, :], in1=dsc[:])
                        if sc == 3:
                            # final value of this o-chunk: stream it out now
                            nc.sync.dma_start(out3[:, o, :], acc[:, o, :])

    nc.compile()
    return nc


def _prepare_inputs(inputs):
    """Build the 8 per-core in_maps from the full problem inputs."""
    x = np.ascontiguousarray(inputs["x"], dtype=np.float32)   # [B, S, D]
    n1 = np.asarray(inputs["norm1_w"], dtype=np.float32)
    n2 = np.asarray(inputs["norm2_w"], dtype=np.float32)
    wq_f = n1[:, None] * np.asarray(inputs["wq"], np.float32)
    wk_f = n1[:, None] * np.asarray(inputs["wk"], np.float32)
    wv_f = n1[:, None] * np.asarray(inputs["wv"], np.float32)
    wo_f = np.ascontiguousarray(np.asarray(inputs["wo"], np.float32))
    wg_f = np.ascontiguousarray(n2[:, None] * np.asarray(inputs["w_gate"], np.float32))
    wu_f = np.ascontiguousarray(n2[:, None] * np.asarray(inputs["w_up"], np.float32))
    wd_f = np.ascontiguousarray(np.asarray(inputs["w_down"], np.float32))
    bq_full = np.asarray(inputs["bq"], np.float32)
    bk_full = np.asarray(inputs["bk"], np.float32)
    bv_full = np.asarray(inputs["bv"], np.float32)
    bo = np.asarray(inputs["bo"], np.float32).reshape(DCH, P).T.copy()
    bg = np.asarray(inputs["b_gate"], np.float32).reshape(FCH, P).T.copy()
    bu = np.asarray(inputs["b_up"], np.float32).reshape(FCH, P).T.copy()
    bd = np.asarray(inputs["b_down"], np.float32).reshape(DCH, P).T.copy()

    bf = ml_dtypes.bfloat16
    cosk, sink = _rope_tables()
    cosk = cosk.astype(bf)
    sink = sink.astype(bf)
    onesb_np = np.ones((P, 1), bf)
    epsv = np.full((P, 1), EPS, np.float32)
    # diagonal 0/1 masks: mask[dd][k, q] = 1 iff k + 128*dd <= q
    kk = np.arange(P)[:, None]
    qq = np.arange(TOK)[None, :]
    maskb = np.stack([(kk + P * dd <= qq) for dd in range(4)], axis=1)
    maskb = np.ascontiguousarray(maskb.astype(bf))

    # pre-tiled weights
    wq_b = np.ascontiguousarray(
        wq_f.astype(bf).reshape(DCH, P, D).transpose(1, 0, 2))  # [p, o, HD*H]
    wk_b = np.ascontiguousarray(
        wk_f.astype(bf).reshape(DCH, P, D).transpose(1, 0, 2))
    wv_b = np.ascontiguousarray(
        wv_f.astype(bf).reshape(DCH, P, D).transpose(1, 0, 2))
    wo_b = np.ascontiguousarray(
        wo_f.astype(bf).reshape(DCH, P, DCH, P).transpose(2, 1, 0, 3))
    f8 = ml_dtypes.float8_e4m3
    # fp8 FFN weights, scaled by WSC=64 into e4m3's sweet spot, pre-tiled
    # with the DoubleRow [p, o-pair, j, f] interleave
    wg_b = np.ascontiguousarray(
        (wg_f * WSC).astype(f8).reshape(8, 2, P, FCH, P)
        .transpose(3, 2, 0, 1, 4))                     # [fb, p, op, j, f]
    wu_b = np.ascontiguousarray(
        (wu_f * WSC).astype(f8).reshape(8, 2, P, FCH, P)
        .transpose(3, 2, 0, 1, 4))
    wd_b = np.ascontiguousarray(
        (wd_f * WSC).astype(f8).reshape(4, 8, 2, P, DCH, P)
        .transpose(0, 4, 3, 1, 2, 5))                  # [sc, o, p, fsp, j, f]

    xT = np.ascontiguousarray(
        x.reshape(B * S, D).T)                                 # [D, B*S]

    in_maps = []
    for c in range(NC):
        hcol = 2 * c * HD                                      # head-col base
        in_maps.append({
            "xT_all": xT,
            "xT_own": np.ascontiguousarray(xT[:, c * TOK:(c + 1) * TOK]),
            "wq": np.ascontiguousarray(wq_b[:, :, hcol:hcol + NH * HD]),
            "wk": np.ascontiguousarray(wk_b[:, :, hcol:hcol + NH * HD]),
            "wv": np.ascontiguousarray(wv_b[:, :, hcol:hcol + NH * HD]),
            "wo": wo_b,
            "wg": wg_b, "wu": wu_b, "wd": wd_b,
            "bq": bq_full[hcol:hcol + NH * HD].reshape(NH, P).T.copy(),
            "bk": bk_full[hcol:hcol + NH * HD].reshape(NH, P).T.copy(),
            "bvb": np.tile(bv_full[hcol:hcol + NH * HD][None, :],
                           (P, 1)).copy(),
            "bo": bo, "bg": bg, "bu": bu, "bd": bd,
            "cosk": cosk, "sink": sink,
            "maskb": maskb, "onesb": onesb_np, "epsv": epsv,
        })
    return in_maps


def kernel(**inputs):
    global LAST_RESULT
    if "nc" not in _CACHE:
        _CACHE["nc"] = _build_program()
    nc = _CACHE["nc"]
    in_maps = _prepare_inputs(inputs)
    trace = bool(int(os.environ.get("BASS_TRACE", "0")))
    res = run_bass_kernel_spmd(nc, in_maps, core_ids=list(range(NC)),
                               trace=trace)
    LAST_RESULT = res
    # assemble: per-core out [D, TOK] -> [D, B*S] -> [B, S, D]
    full = np.concatenate([res.results[c]["out"] for c in range(NC)], axis=1)
    return np.ascontiguousarray(full.T).reshape(B, S, D)


if __name__ == "__main__":
    print("import as module; use kernel(**inputs)")
